# revision 1
# baseline (speedup 1.0000x reference)
"""AttnBlock (GroupNorm + single-head attention + proj + residual) on 8 trn2 cores.

Sharding: core = (image b, query-half h).  Each core gets the full 4096-pixel
image (queries permuted to rows 0:2048), computes GroupNorm + K/V for the whole
image, attention/proj for its 2048 queries, and writes a (2048, 512) shard.

Math folding done on host (all exact, fp32):
  h = gn(x)*gamma + beta ; q = h@wq+bq ; ... ; out = x + attn@wp + bp
  => with xn = (x-mu_g)*rstd_g (pure normalize):
     q' = (xn@wq_f + bq_f)          wq_f = diag(gamma)@wq/sqrt(C), bq_f = (beta@wq+bq)/sqrt(C)
     k  = (xn@wk_f + bk_f)          wk_f = diag(gamma)@wk,        bk_f = beta@wk+bk
     v' = xn@wv_f (bias dropped)    wv_f = diag(gamma)@wv
     out = x + softmax(q'k^T)@v'@wp + bfin,   bfin = (beta@wv+bv)@wp + bp
  (softmax rows sum to 1, so the v-bias passes through attention unchanged.)
"""

import sys

sys.path.insert(0, "/opt/trn_rl_repo")

import numpy as np
import ml_dtypes

import concourse.bass as bass
import concourse.tile as tile
from concourse import mybir
from concourse.masks import make_identity

F32 = mybir.dt.float32
BF16 = mybir.dt.bfloat16
FP8 = mybir.dt.float8e4
USE_FP8 = True
W8SCALE = 64.0
AF = mybir.ActivationFunctionType
ALU = mybir.AluOpType
AX = mybir.AxisListType

PIX = 4096          # 64*64 pixels per image
QPIX = 2048         # queries per core
C = 512             # channels
NCH = 4             # channel chunks of 128
NQT = QPIX // 128   # 16 query tiles
NKB = PIX // 128    # 32 key blocks
NKT = PIX // 512    # 8 key tiles (psum-bank sized)
NPT = PIX // 128    # 32 pixel tiles
EPS = 1e-5

_CACHED = {}


def build_program(spill=True):
    nc = bass.Bass()

    x = nc.dram_tensor("x", [PIX, C], F32, kind="ExternalInput").ap()
    xt_in = nc.dram_tensor("xt", [C, PIX], BF16, kind="ExternalInput").ap()
    WDT = FP8 if USE_FP8 else BF16
    wq = nc.dram_tensor("wq", [C, C], WDT, kind="ExternalInput").ap()
    wk = nc.dram_tensor("wk", [C, C], WDT, kind="ExternalInput").ap()
    wv = nc.dram_tensor("wv", [C, C], WDT, kind="ExternalInput").ap()
    wp = nc.dram_tensor("wp", [C, C], BF16, kind="ExternalInput").ap()
    bq = nc.dram_tensor("bq", [C], F32, kind="ExternalInput").ap()
    bk = nc.dram_tensor("bk", [C], F32, kind="ExternalInput").ap()
    bfin = nc.dram_tensor("bfin", [C], F32, kind="ExternalInput").ap()
    gmask = nc.dram_tensor("gmask", [128, 128], F32, kind="ExternalInput").ap()
    out = nc.dram_tensor("out", [QPIX, C], F32, kind="ExternalOutput").ap()

    with tile.TileContext(nc) as tc:
        with (
            tc.tile_pool(name="singles", bufs=1) as singles,
            tc.tile_pool(name="big", bufs=1) as big,
            tc.tile_pool(name="work", bufs=3) as work,
            tc.tile_pool(name="stats", bufs=2) as stats,
        ):
            # ---- constants / weights ----
            ident_b = singles.tile([128, 128], BF16, tag="idb")
            make_identity(nc, ident_b)
            ident_8 = singles.tile([128, 128], FP8, tag="id8")
            make_identity(nc, ident_8)
            PDT = FP8 if USE_FP8 else BF16
            ident_p = ident_8 if USE_FP8 else ident_b
            gmask_sb = singles.tile([128, 128], F32, tag="gmask")
            nc.gpsimd.dma_start(out=gmask_sb, in_=gmask)
            eps_sb = singles.tile([128, 1], F32, tag="eps")
            nc.vector.memset(eps_sb, EPS)
            # constant exp bias: keeps exp() in fp8-e4m3 range for the P cast;
            # cancels exactly in the softmax normalization
            ebias_sb = singles.tile([128, 1], F32, tag="ebias")
            nc.vector.memset(ebias_sb, -2.772589)

            w_sb = {}
            for name, ap in (("wq", wq), ("wk", wk), ("wv", wv), ("wp", wp)):
                t = singles.tile([128, NCH, C], ap.dtype, tag=name)
                nc.gpsimd.dma_start(
                    out=t, in_=ap.rearrange("(ci p) co -> p ci co", p=128)
                )
                w_sb[name] = t
            # per-chunk partition-major bias columns: b_sb[:, ci] = b[ci*128+p]
            bq_sb = singles.tile([128, NCH], F32, tag="bq")
            nc.gpsimd.dma_start(out=bq_sb, in_=bq.rearrange("(c p) -> p c", p=128))
            bk_sb = singles.tile([128, NCH], F32, tag="bk")
            nc.gpsimd.dma_start(out=bk_sb, in_=bk.rearrange("(c p) -> p c", p=128))
            # bfin broadcast across partitions: [128, 512]
            bfin_bc = singles.tile([128, C], F32, tag="bfin")
            nc.gpsimd.dma_start(
                out=bfin_bc,
                in_=bass.AP(tensor=bfin.tensor, offset=bfin.offset,
                            ap=[[0, 128], [1, C]]),
            )

            # ---- persistent big tensors ----
            # xh: x transposed to [ch, pix] (bf16); normalized into h (PDT)
            xh = big.tile([128, NCH, PIX], BF16, tag="xh")    # 32KB/part
            if USE_FP8:
                h = big.tile([128, NCH, PIX], PDT, tag="h", name="h")
            else:
                h = xh
            kT = big.tile([128, NCH, PIX], PDT, tag="kT")
            qT = big.tile([128, NCH, QPIX], PDT, tag="qT")
            V = big.tile([128, NKB, C], PDT, tag="V")
            # residual x for this core's queries, loaded once: [p, qt, c]
            xq_all = big.tile([128, NQT, C], F32, tag="xq")   # 32KB/part
            nc.gpsimd.dma_start(
                out=xq_all,
                in_=x[0:QPIX, :].rearrange("(t p) c -> p t c", p=128),
            )
            bfin_rep = bass.AP(tensor=bfin_bc.tensor, offset=bfin_bc.offset,
                               ap=[bfin_bc.ap[0], [0, NQT], bfin_bc.ap[1]])
            nc.vector.tensor_add(xq_all, xq_all, bfin_rep)

            bn6 = stats.tile([128, NCH, NKT, 6], F32, tag="bn6")

            # ---- phase 1: load x^T (host-transposed), accumulate bn stats ----
            with tc.tile_pool(name="psumA", bufs=2, space="PSUM") as psA:
                for ci in range(NCH):
                    for s4 in range(4):
                        nc.gpsimd.dma_start(
                            out=xh[:, ci, s4 * 1024:(s4 + 1) * 1024],
                            in_=xt_in[ci * 128:(ci + 1) * 128,
                                      s4 * 1024:(s4 + 1) * 1024],
                        )
                for ci in range(NCH):
                    for sg in range(NKT):
                        nc.vector.bn_stats(
                            out=bn6[:, ci, sg, :],
                            in_=xh[:, ci, sg * 512:(sg + 1) * 512],
                        )

                # ---- phase 2: group stats + normalize -> hT (bf16) ----
                for ci in range(NCH):
                    mv = stats.tile([128, 2], F32, tag="mv")
                    nc.vector.bn_aggr(out=mv, in_=bn6[:, ci, :, :])
                    me = stats.tile([128, 2], F32, tag="me")
                    nc.vector.tensor_copy(me[:, 0:1], mv[:, 0:1])
                    nc.vector.tensor_mul(me[:, 1:2], mv[:, 0:1], mv[:, 0:1])
                    nc.vector.tensor_add(me[:, 1:2], me[:, 1:2], mv[:, 1:2])
                    gps = psA.tile([128, 2], F32, tag="gs")
                    nc.tensor.matmul(gps, gmask_sb, me, start=True, stop=True)
                    gst = stats.tile([128, 2], F32, tag="gst")
                    nc.vector.tensor_copy(gst, gps)
                    vg = stats.tile([128, 1], F32, tag="vg")
                    nc.vector.tensor_mul(vg, gst[:, 0:1], gst[:, 0:1])
                    nc.vector.tensor_sub(vg, gst[:, 1:2], vg)
                    rstd = stats.tile([128, 1], F32, tag="rstd")
                    nc.scalar.activation(out=rstd, in_=vg, func=AF.Sqrt,
                                         bias=eps_sb, scale=1.0)
                    nc.vector.reciprocal(out=rstd, in_=rstd)
                    for ns in range(NKT):
                        nc.vector.tensor_scalar(
                            out=h[:, ci, ns * 512:(ns + 1) * 512],
                            in0=xh[:, ci, ns * 512:(ns + 1) * 512],
                            scalar1=gst[:, 0:1], scalar2=rstd,
                            op0=ALU.subtract, op1=ALU.mult,
                        )

            # ---- phase 3: qT, kT, V gemms ----
            DR = mybir.MatmulPerfMode.DoubleRow if USE_FP8 else None
            DS = 1.0 / W8SCALE if USE_FP8 else 1.0
            with tc.tile_pool(name="psumC", bufs=3, space="PSUM") as psC:
                def lin_gemm(wname, dst, bias_sb, nt, co, idx):
                    ps = psC.tile([128, 512], F32, tag="lin", name=f"lin{idx}")
                    if USE_FP8:
                        for u in range(2):
                            nc.tensor.matmul(
                                ps,
                                w_sb[wname][:, 2 * u:2 * u + 2,
                                            co * 128:(co + 1) * 128],
                                h[:, 2 * u:2 * u + 2,
                                  nt * 512:(nt + 1) * 512],
                                start=(u == 0), stop=(u == 1),
                                perf_mode=DR,
                            )
                    else:
                        for ci in range(NCH):
                            nc.tensor.matmul(
                                ps,
                                w_sb[wname][:, ci, co * 128:(co + 1) * 128],
                                h[:, ci, nt * 512:(nt + 1) * 512],
                                start=(ci == 0), stop=(ci == NCH - 1),
                            )
                    if idx % 2 == 0:
                        nc.scalar.activation(
                            out=dst[:, co, nt * 512:(nt + 1) * 512], in_=ps,
                            func=AF.Identity, scale=DS,
                            bias=bias_sb[:, co:co + 1],
                        )
                    else:
                        nc.vector.tensor_scalar(
                            out=dst[:, co, nt * 512:(nt + 1) * 512],
                            in0=ps, scalar1=DS, scalar2=bias_sb[:, co:co + 1],
                            op0=ALU.mult, op1=ALU.add,
                        )

                idx = 0
                for nt in range(NKT):
                    for co in range(NCH):
                        lin_gemm("wk", kT, bk_sb, nt, co, idx); idx += 1
                for nt in range(QPIX // 512):
                    for co in range(NCH):
                        lin_gemm("wq", qT, bq_sb, nt, co, idx); idx += 1
                for kb in range(NKB):
                    ps = psC.tile([128, 512], F32, tag="lin")
                    if USE_FP8:
                        for u in range(2):
                            nc.tensor.matmul(
                                ps,
                                h[:, 2 * u:2 * u + 2, kb * 128:(kb + 1) * 128],
                                w_sb["wv"][:, 2 * u:2 * u + 2, :],
                                start=(u == 0), stop=(u == 1),
                                perf_mode=DR,
                            )
                    else:
                        for ci in range(NCH):
                            nc.tensor.matmul(
                                ps,
                                h[:, ci, kb * 128:(kb + 1) * 128],
                                w_sb["wv"][:, ci, :],
                                start=(ci == 0), stop=(ci == NCH - 1),
                            )
                    if USE_FP8:
                        if kb % 2 == 0:
                            nc.scalar.mul(out=V[:, kb, :], in_=ps, mul=DS)
                        else:
                            nc.vector.tensor_scalar_mul(
                                out=V[:, kb, :], in0=ps, scalar1=DS)
                    else:
                        nc.vector.tensor_copy(V[:, kb, :], ps)

            # ---- phase 4: attention + proj + residual, per query tile ----
            with (
                tc.tile_pool(name="psumB", bufs=2, space="PSUM") as psB,
                tc.tile_pool(name="attn", bufs=3) as attn,
            ):
                SXS = DS * DS  # undo the two fp8 weight scales on S? (none: q,k already descaled)
                for qt in range(NQT):
                    P = attn.tile([128, PIX], BF16, tag="P")
                    sums = attn.tile([128, NKT], F32, tag="sums")
                    for kt in range(NKT):
                        sps = psB.tile([128, 512], F32, tag="s")
                        if USE_FP8:
                            for u in range(2):
                                nc.tensor.matmul(
                                    sps,
                                    qT[:, 2 * u:2 * u + 2,
                                       qt * 128:(qt + 1) * 128],
                                    kT[:, 2 * u:2 * u + 2,
                                       kt * 512:(kt + 1) * 512],
                                    start=(u == 0), stop=(u == 1),
                                    perf_mode=DR,
                                )
                        else:
                            for ci in range(NCH):
                                nc.tensor.matmul(
                                    sps,
                                    qT[:, ci, qt * 128:(qt + 1) * 128],
                                    kT[:, ci, kt * 512:(kt + 1) * 512],
                                    start=(ci == 0), stop=(ci == NCH - 1),
                                )
                        # constant bias keeps exp() inside fp8-e4m3 range for
                        # the P^T cast; cancels in the softmax normalization
                        nc.scalar.activation(
                            out=P[:, kt * 512:(kt + 1) * 512], in_=sps,
                            func=AF.Exp, scale=1.0, bias=ebias_sb,
                            accum_out=sums[:, kt:kt + 1],
                        )
                    rcp = attn.tile([128, 1], F32, tag="rcp")
                    nc.vector.reduce_sum(out=rcp, in_=sums, axis=AX.X)
                    nc.vector.reciprocal(out=rcp, in_=rcp)

                    pvps = psB.tile([128, 512], F32, tag="pv")
                    GB = 8  # transposed P blocks per psum bank batch
                    for g in range(NKB // GB):
                        ptps = psB.tile([128, GB * 128], BF16, tag="pt")
                        for j in range(GB):
                            kb = g * GB + j
                            nc.tensor.transpose(
                                ptps[:, j * 128:(j + 1) * 128],
                                P[:, kb * 128:(kb + 1) * 128],
                                ident_b,
                            )
                        ptsb = work.tile([128, GB, 128], PDT, tag="ptsb")
                        nc.vector.tensor_copy(
                            ptsb, ptps.rearrange("p (j k) -> p j k", j=GB))
                        if USE_FP8:
                            for j in range(GB // 2):
                                kb = g * GB + 2 * j
                                nc.tensor.matmul(
                                    pvps,
                                    ptsb[:, 2 * j:2 * j + 2, :],
                                    V[:, kb:kb + 2, :],
                                    start=(kb == 0), stop=(kb == NKB - 2),
                                    perf_mode=DR,
                                )
                        else:
                            for j in range(GB):
                                kb = g * GB + j
                                nc.tensor.matmul(
                                    pvps,
                                    ptsb[:, j, :],
                                    V[:, kb, :],
                                    start=(kb == 0), stop=(kb == NKB - 1),
                                )
                    ao = work.tile([128, C], BF16, tag="ao")
                    nc.vector.tensor_scalar_mul(out=ao, in0=pvps, scalar1=rcp)

                    atps = psB.tile([128, 512], BF16, tag="pt")
                    for ci in range(NCH):
                        nc.tensor.transpose(
                            atps[:, ci * 128:(ci + 1) * 128],
                            ao[:, ci * 128:(ci + 1) * 128],
                            ident_b,
                        )
                    atsb = work.tile([128, 512], BF16, tag="ptsb")
                    nc.vector.tensor_copy(atsb, atps)

                    pjps = psB.tile([128, 512], F32, tag="pv")
                    for ci in range(NCH):
                        nc.tensor.matmul(
                            pjps,
                            atsb[:, ci * 128:(ci + 1) * 128],
                            w_sb["wp"][:, ci, :],
                            start=(ci == 0), stop=(ci == NCH - 1),
                        )
                    o_sb = work.tile([128, C], F32, tag="osb")
                    nc.vector.tensor_add(o_sb, pjps, xq_all[:, qt, :])
                    nc.gpsimd.dma_start(
                        out=out[qt * 128:(qt + 1) * 128, :], in_=o_sb
                    )
    if spill:
        _spill_excess_waits(nc)
    return nc


def _spill_excess_waits(nc):
    """Walrus enforces tight per-instruction sync-wait slot limits (1 for
    most opcodes, 2 for EventSemaphore).  Tile's sem assignment occasionally
    emits more at multi-producer joins; spill the excess onto same-engine
    EventSemaphore ops inserted immediately before the offender."""
    n = 0
    for fn in nc.m.functions:
        for bb in fn.blocks:
            out_insts = []
            changed = False
            for inst in bb.instructions:
                si = inst.sync_info
                waits = list(si.on_wait) if si is not None and si.on_wait else []
                cap = 2 if inst.__class__.__name__ == "InstEventSemaphore" else 1
                if len(waits) > cap:
                    keep = waits[-cap:]
                    excess = waits[:-cap]
                    for j in range(0, len(excess), 2):
                        n += 1
                        es = mybir.InstEventSemaphore(
                            name=f"W-spill-{n}",
                            engine=inst.engine,
                            ins=[], outs=[],
                            sync_info=mybir.SyncInfo(
                                on_wait=excess[j:j + 2], on_update=[]
                            ),
                        )
                        out_insts.append(es)
                    si.on_wait = keep
                    changed = True
                out_insts.append(inst)
            if changed:
                bb.instructions = out_insts
    return n


def _prepare(x, gamma, beta, wq, bq, wk, bk, wv, bv, wp, bp):
    f32 = np.float32
    x = np.asarray(x, f32)
    gamma = np.asarray(gamma, f32)
    beta = np.asarray(beta, f32)
    scale = f32(1.0 / np.sqrt(C))
    wq_f = (gamma[:, None] * np.asarray(wq, f32)) * scale
    bq_f = (beta @ np.asarray(wq, f32) + np.asarray(bq, f32)) * scale
    wk_f = gamma[:, None] * np.asarray(wk, f32)
    bk_f = beta @ np.asarray(wk, f32) + np.asarray(bk, f32)
    wv_f = gamma[:, None] * np.asarray(wv, f32)
    bv_f = beta @ np.asarray(wv, f32) + np.asarray(bv, f32)
    wp_f = np.asarray(wp, f32)
    bfin = bv_f @ wp_f + np.asarray(bp, f32)
    gm = (np.kron(np.eye(8, dtype=f32), np.ones((16, 16), f32)) / 16.0)
    bf = ml_dtypes.bfloat16
    if USE_FP8:
        f8 = ml_dtypes.float8_e4m3
        common = dict(
            wq=(wq_f * W8SCALE).astype(f8), wk=(wk_f * W8SCALE).astype(f8),
            wv=(wv_f * W8SCALE).astype(f8),
            wp=wp_f.astype(bf), bq=bq_f.astype(f32), bk=bk_f.astype(f32),
            bfin=bfin.astype(f32), gmask=gm,
        )
    else:
        common = dict(
            wq=wq_f.astype(bf), wk=wk_f.astype(bf), wv=wv_f.astype(bf),
            wp=wp_f.astype(bf), bq=bq_f.astype(f32), bk=bk_f.astype(f32),
            bfin=bfin.astype(f32), gmask=gm,
        )
    in_maps = []
    for b in range(4):
        xb = np.ascontiguousarray(x[b].reshape(PIX, C))
        for h in range(2):
            xp = xb if h == 0 else np.concatenate([xb[QPIX:], xb[:QPIX]])
            xp = np.ascontiguousarray(xp)
            xtp = np.ascontiguousarray(xp.T).astype(ml_dtypes.bfloat16)
            in_maps.append(dict(common, x=xp, xt=xtp))
    return in_maps


def kernel(x, gamma, beta, wq, bq, wk, bk, wv, bv, wp, bp, _trace=False):
    from concourse.bass_utils import run_bass_kernel_spmd

    if "nc" not in _CACHED:
        _CACHED["nc"] = build_program()
    nc = _CACHED["nc"]
    in_maps = _prepare(x, gamma, beta, wq, bq, wk, bk, wv, bv, wp, bp)
    res = run_bass_kernel_spmd(nc, in_maps, list(range(8)), trace=_trace)
    _CACHED["last_results"] = res
    out = np.empty((4, PIX, C), np.float32)
    for core in range(8):
        b, h = divmod(core, 2)
        out[b, h * QPIX:(h + 1) * QPIX] = res.results[core]["out"]
    return out.reshape(4, 64, 64, C)

